# revision 32
# baseline (speedup 1.0000x reference)
"""Bi-directional correlation cost volume on 8 Trainium2 NeuronCores.

Strategy (data-parallel over batch, one batch element per core):
  - Host casts both feature maps to bf16 (rel err ~3e-3, gate is 2e-2).
    The PE computes the Gram band G[u, x] = sum_c L[c,h,u] * R[c,h,x]
    at bf16 rate, 4x h-quadrant row-tiled (K=32 each) via tile_position.
  - The u-range [0,320) is covered by chunks of U in {128, 64}. U=64
    chunks pack TWO h-rows per PSUM tile (partitions 0-63 / 64-127 via
    tile_position col-tile 0/64), so every staging tile is a full 128
    partitions. That (a) halves those chunks' drain columns and (b)
    makes the whole quadrant's band one dense [128, FREE] SBUF tile.
  - Stores: ONE ~4.1MB DMA per quadrant (4/rep). Measured HWDGE-ring
    cost is ~1us fixed per dma_start + bytes at ~351 GB/s; the V1
    design's 24 stores/rep paid ~24us/rep of fixed cost alone.
  - KROT=2: input partitions hold (h%4, c) so consecutive matmuls cycle
    all 4 PE row-groups; with bank-interleaved PSUM slot offsets
    (concurrent row tiles must hit distinct banks) the PE subarray
    tiles overlap ~1.5x, dropping PE from 54.9us serial to ~37us -
    below the store wall.
  - PSUM (f32) drains to SBUF as bf16 with the 1/C scale fused, batched
    GB=4 matmuls per instruction, split across DVE and ACT by greedy
    load balance (~2.16 cols/ns combined).
  - Default cover (KCH=k5f8): five U=64 chunks (52736 band elems per
    h-row), with the two edge chunks staged as fp8e4 and decoded on
    host: 30.8% of elements at ~2.6% RMS quantization -> rel err
    1.556e-2 vs the 2e-2 gate, and staged bytes drop to 14.28MB/rep
    (27880 B/partition quadrant stores, under the ~33KB DMA-rate
    cliff). Measured 49.0us/rep, PE/drain/store co-walled; history:
    77.9 (V1, 24 stores) -> 56.6 (k5p single-store) -> 52.9 (KROT=2)
    -> 50.6 (fp8 edges) -> 49.0 (UNROLL=32 For_i amortization).
  - The cost volume out[d, x] = G[x -/+ d, x] is a shear of the band;
    host extracts the diagonals with one vectorized gather per batch.
"""

import os

import numpy as np

B, C, H, WIMG, D = 8, 32, 160, 320, 64
HQ = H // 4      # 40 h-rows per PE quadrant

# (u0, U, xw0, W, pack): u-chunk start/size, x-window start/size,
# h-rows packed per PSUM partition-group (pack = 128 // U).
_KCH = os.environ.get("KCH", "k5f8")
if _KCH == "k3p":
    CHUNKS = [(0, 128, 0, 191, 1), (128, 64, 65, 190, 2),
              (192, 128, 129, 191, 1)]
elif _KCH == "k4p":
    CHUNKS = [(0, 128, 0, 191, 1), (128, 64, 65, 190, 2),
              (192, 64, 129, 190, 2), (256, 64, 193, 127, 2)]
elif _KCH == "k4f8":
    # k4p geometry (698 PE cols/h-row vs k5p's 824 - the PE stream is
    # the wall once fp8 pays for the bytes) with chunks c1+c3 staged
    # fp8e4: 35.7% of elements at ~2.6% RMS -> total rel err ~1.7e-2;
    # 14.96MB/rep staged, 29220 B/partition quadrant stores.
    CHUNKS = [(0, 128, 0, 191, 1), (128, 64, 65, 190, 2),
              (192, 64, 129, 190, 2), (256, 64, 193, 127, 2)]
elif _KCH == "k5f8":
    # k5p geometry, but the two edge chunks staged as fp8e4 (the host
    # decodes them back to f32): 30.8% of band elements at ~2.6% RMS
    # quantization -> total rel err ~1.5e-2 (gate 2e-2), staged bytes
    # drop 15.4% to 27880 B/partition per quadrant store.
    CHUNKS = [(0, 64, 0, 127, 2), (64, 64, 1, 190, 2),
              (128, 64, 65, 190, 2), (192, 64, 129, 190, 2),
              (256, 64, 193, 127, 2)]
elif _KCH == "k6m":
    # four U=64 + two U=32 chunks: 51712 elems/h-row (-2% vs k5p),
    # 32320 B/partition stores; PE 61.3us serial needs KROT=2.
    CHUNKS = [(0, 64, 0, 127, 2), (64, 64, 1, 190, 2),
              (128, 64, 65, 190, 2), (192, 64, 129, 190, 2),
              (256, 32, 193, 127, 4), (288, 32, 225, 95, 4)]
elif _KCH == "k10":
    # all-U=32 cover, pack=4 (needs KROT=2 PE tile concurrency to keep
    # the 92.8us serial PE stream under the ~47us store wall): 44544
    # band elems/h-row -> 14.25MB/rep, stores 27840 B/partition.
    CHUNKS = [(0, 32, 0, 95, 4), (32, 32, 0, 127, 4),
              (64, 32, 1, 158, 4), (96, 32, 33, 158, 4),
              (128, 32, 65, 158, 4), (160, 32, 97, 158, 4),
              (192, 32, 129, 158, 4), (224, 32, 161, 158, 4),
              (256, 32, 193, 127, 4), (288, 32, 225, 95, 4)]
else:
    # all-U=64 cover: fewest staged bytes (52736 elems/h-row) AND a
    # per-quadrant store of 32960 B/partition, which sits below the
    # ~33KB/partition DMA-rate cliff (351 GB/s vs 306 above it).
    CHUNKS = [(0, 64, 0, 127, 2), (64, 64, 1, 190, 2),
              (128, 64, 65, 190, 2), (192, 64, 129, 190, 2),
              (256, 64, 193, 127, 2)]

# staging free-dim offsets: chunk ci occupies [FOFF, FOFF+(HQ//pack)*W)
# elements; F8[ci] chunks are staged as fp8e4 (1B), the rest bf16 (2B).
F8 = {"k5f8": [True, False, False, False, True],
      "k4f8": [False, True, False, True]}.get(_KCH, [False] * len(CHUNKS))
ESZ = [1 if f else 2 for f in F8]
FOFFS, FREE = [], 0          # element offsets (host-side f32 array)
BOFFS, BTOT = [], 0          # byte offsets (device staging layout)
for _ci, (_u0, _U, _x0, _W, _p) in enumerate(CHUNKS):
    FOFFS.append(FREE)
    FREE += (HQ // _p) * _W
    BOFFS.append(BTOT)
    BTOT += (HQ // _p) * _W * ESZ[_ci]

_CACHE = {}

# tuning knobs (env-overridable for experiments)
GB = int(os.environ.get("KGB", "4"))         # matmuls per PSUM tile/copy
# PE row-band rotation (partition (h%4, c)) so consecutive matmuls hit
# distinct PE row-groups -> subarray tile concurrency (~1.5x measured).
# Concurrent row tiles MUST write distinct PSUM banks: KROT=2 keeps the
# standard 4x2-bank PSUM tiles but bank-interleaves slot offsets
# (0,512,256,768) so every same-col-half pair in the concurrency window
# differs in bank. (KROT=1, a 2x4-bank spread, starves PSUM
# double-buffering and is slower; KROT=0 is the serial-PE layout.)
KROT = int(os.environ.get("KROT", "2"))
KPE2 = int(os.environ.get("KPE2", "0"))      # issue each matmul twice (probe)
PS_BUFS = int(os.environ.get("KPSB", "4"))   # PSUM pool buffers
ST_BUFS = int(os.environ.get("KSTB", "3"))   # staging pool buffers
UNROLL = int(os.environ.get("KUNR", "32"))   # reps per For_i iteration
KF8S = int(os.environ.get("KF8S", "0"))      # force fp8 drains onto ACT
NDMAQ = int(os.environ.get("KDMA", "1"))     # store-DMA queues


def _get_nc(reps=1, hw_loop=False):
    """reps identical kernel bodies; with hw_loop, a For_i loop of
    reps//UNROLL iterations around an UNROLL-times unrolled body (constant
    NEFF size, so huge rep counts stay compilable - used for timing)."""
    key = ("nc", reps, GB, PS_BUFS, ST_BUFS, _KCH, hw_loop, UNROLL, NDMAQ,
           KROT, KPE2, KF8S)
    if key in _CACHE:
        return _CACHE[key]
    import concourse.bacc as bacc
    import concourse.tile as tile
    from concourse import mybir

    f32 = mybir.dt.float32
    bf16 = mybir.dt.bfloat16
    nc = bacc.Bacc("TRN2", target_bir_lowering=False, debug=False)
    r_in = nc.declare_dram_parameter("r_in", [C, H, WIMG], bf16, isOutput=False)
    l_in = nc.declare_dram_parameter("l_in", [C, H, WIMG], bf16, isOutput=False)
    u8 = mybir.dt.uint8
    f8 = mybir.dt.float8e4
    stag = nc.declare_dram_parameter("stag", [4, 128 * BTOT], u8,
                                     isOutput=True)

    # KROT=1: 2 x 4-bank tiles = full PSUM; KROT=2: standard 4 x 2-bank
    ps_bufs = 2 if KROT == 1 else PS_BUFS
    with tile.TileContext(nc) as tc:
        with tc.tile_pool(name="inp", bufs=1) as inp_pool, \
             tc.tile_pool(name="ps", bufs=ps_bufs, space="PSUM") as ps_pool, \
             tc.tile_pool(name="st", bufs=ST_BUFS) as st_pool:
            Lsb = inp_pool.tile([128, HQ * WIMG], bf16, tag="L")
            Rsb = inp_pool.tile([128, HQ * WIMG], bf16, tag="R")
            if KROT:
                # partition (h%4, c) holds h-row h at free slot h//4: the
                # mm stream then cycles all 4 PE row-groups (tile
                # concurrency), while the PSUM/staging layout is unchanged.
                for s in range(4):
                    nc.sync.dma_start(
                        Lsb[32 * s:32 * (s + 1), :].rearrange(
                            "p (hh x) -> p hh x", x=WIMG),
                        l_in[:, s::4, :])
                    nc.sync.dma_start(
                        Rsb[32 * s:32 * (s + 1), :].rearrange(
                            "p (hh x) -> p hh x", x=WIMG),
                        r_in[:, s::4, :])
            else:
                # partition (q, c) holds h-rows [40q, 40q+40) of channel c
                for q in range(4):
                    nc.sync.dma_start(
                        Lsb[32 * q:32 * (q + 1), :],
                        l_in[:, HQ * q:HQ * (q + 1), :].rearrange(
                            "c hh x -> c (hh x)"),
                    )
                    nc.sync.dma_start(
                        Rsb[32 * q:32 * (q + 1), :],
                        r_in[:, HQ * q:HQ * (q + 1), :].rearrange(
                            "c hh x -> c (hh x)"),
                    )
            # greedy copy-engine balance: projected busy ns per engine
            # (per-instruction init: ACT 2*222cyc/2, DVE 2*120cyc/2 busy)
            eng_ns = {"v": 0.0, "s": 0.0}
            cyc = {"v": 1.0 / 0.96, "s": 1.0 / 1.2}
            init = {"v": 120 * cyc["v"], "s": 222 * cyc["s"]}

            def mm1(ps, q, g, kslot, ci):
                u0, U, xw0, W, pack = CHUNKS[ci]
                for j in range(pack):
                    hh = g * pack + j
                    if KROT:
                        s, fr = hh % 4, 10 * q + hh // 4
                    else:
                        s, fr = q, hh
                    for _rep in range(1 + KPE2):
                        nc.tensor.matmul(
                            ps[U * j:U * j + U,
                               256 * kslot:256 * kslot + W],
                            Lsb[32 * s:32 * (s + 1),
                                fr * WIMG + u0:fr * WIMG + u0 + U],
                            Rsb[32 * s:32 * (s + 1),
                                fr * WIMG + xw0:fr * WIMG + xw0 + W],
                            start=True, stop=True,
                            tile_position=(32 * s, U * j),
                        )

            # With KROT, group a of a batch uses PSUM slot perm(a) so that
            # consecutive (concurrently executing) row-rotated matmuls hit
            # distinct PSUM banks: slot(a) = (a%4)*2 + a//4 for nb=8 (banks
            # 0,1,2,3,0,1,2,3), slot(a) = 2a for nb<=4 (banks 0..3).
            def slot_of(a, nb):
                if not KROT:
                    return a
                if KROT == 2:
                    # bank-interleaved slots within a standard 2-bank tile:
                    # offsets (0, 512, 256, 768) f32 -> banks 0,1,0,1 so any
                    # two same-col-half mms within the concurrency window
                    # differ in bank (same-bank pairs are same-tile-position
                    # and serialize on weights anyway).
                    assert nb in (2, 4)
                    return (a % 2) * 2 + a // 2 if nb == 4 else 2 * a
                return (a % 4) * 2 + a // 4 if nb == 8 else 2 * a

            def drain(ps, sb, g0, nb, ci):
                _, U, _, W, pack = CHUNKS[ci]
                esz = ESZ[ci]
                dt_out = f8 if F8[ci] else bf16

                def dview(ng_):
                    b0 = BOFFS[ci] + g0 * W * esz
                    return sb[:, b0:b0 + ng_ * W * esz].bitcast(dt_out)

                if not KROT:
                    src = ps[:, :256 * nb].rearrange(
                        "u (g c) -> u g c", g=nb)[:, :, :W]
                    dst = dview(nb).rearrange("u (g w) -> u g w", g=nb)
                elif KROT == 2 and nb == 4:
                    # slot offset = (a%2)*512 + (a//2)*256: a = 2*hi + lo
                    src = ps[:, :1024].rearrange(
                        "u (lo hi c) -> u hi lo c", lo=2, hi=2)[:, :, :, :W]
                    dst = dview(4).rearrange("u (hi lo w) -> u hi lo w",
                                             hi=2, lo=2)
                elif KROT == 2:
                    # tail: slots (0, 2) -> offsets (0, 512)
                    assert nb == 2
                    src = ps[:, :1024].rearrange(
                        "u (lo c) -> u lo c", lo=2)[:, :, :W]
                    dst = dview(2).rearrange("u (g w) -> u g w", g=2)
                elif nb == 8:
                    # slot offset = (a%4)*512 + (a//4)*256: a = 4*hi + lo
                    src = ps[:, :2048].rearrange(
                        "u (lo hi c) -> u hi lo c", lo=4, hi=2)[:, :, :, :W]
                    dst = dview(8).rearrange("u (hi lo w) -> u hi lo w",
                                             hi=2, lo=4)
                else:
                    assert nb <= 4
                    src = ps[:, :512 * nb].rearrange(
                        "u (g c) -> u g c", g=nb)[:, :, :W]
                    dst = dview(nb).rearrange("u (g w) -> u g w", g=nb)
                cost = {e: nb * W * cyc[e] + init[e] for e in ("v", "s")}
                if KF8S and F8[ci]:
                    e = "s"
                else:
                    e = min(("v", "s"), key=lambda x: eng_ns[x] + cost[x])
                eng_ns[e] += cost[e]
                if e == "s":
                    nc.scalar.mul(dst, src, 1.0 / C)
                else:
                    nc.vector.tensor_scalar_mul(dst, src, 1.0 / C)

            # 1: SP HWDGE only; 2: +ACT HWDGE; 3: +Pool SWDGE (idle engine,
            # parallel ring - measured ~3us/rep faster than SP alone)
            dmaq = {1: [nc.sync], 2: [nc.sync, nc.scalar],
                    3: [nc.sync, nc.gpsimd]}[NDMAQ]
            store_ct = [0]

            def store(sb, q):
                eng = dmaq[store_ct[0] % len(dmaq)]
                store_ct[0] += 1
                eng.dma_start(
                    stag[q].rearrange("(u k) -> u k", u=128), sb[:, :])

            gb_eff = GB
            ps_w = 2048 if KROT == 1 else 256 * GB  # KROT=1: 4-bank spread

            def rep_body():
                for q in range(4):
                    sb = st_pool.tile([128, BTOT], u8, tag="sb")
                    for ci, (u0, U, xw0, W, pack) in enumerate(CHUNKS):
                        ng = HQ // pack
                        for g0 in range(0, ng, gb_eff):
                            nb = min(gb_eff, ng - g0)
                            ps = ps_pool.tile([128, ps_w], f32,
                                              tag="ps")
                            for k in range(nb):
                                mm1(ps, q, g0 + k, slot_of(k, nb), ci)
                            drain(ps, sb, g0, nb, ci)
                    store(sb, q)

            if hw_loop:
                assert reps % UNROLL == 0
                with tc.For_i(0, reps // UNROLL) as _iv:
                    for _ in range(UNROLL):
                        rep_body()
            else:
                for _ in range(reps):
                    rep_body()
    nc.compile()
    _CACHE[key] = nc
    return nc


def _gather_idx():
    """GIDX[p, h, x]: flat index into stag.ravel() for output plane p."""
    if "idx" in _CACHE:
        return _CACHE["idx"]
    P_ = np.arange(2 * D)[:, None, None]
    dts = np.where(P_ < D, P_, -(P_ - D))  # signed disparity per plane
    X = np.arange(WIMG)[None, None, :]
    u = np.clip(X - dts, 0, WIMG - 1)      # [2D, 1, W]
    ci = np.zeros(u.shape, dtype=np.int64)
    for i in range(1, len(CHUNKS)):
        ci += (u >= CHUNKS[i][0])
    u0 = np.choose(ci, [c[0] for c in CHUNKS])
    xw0 = np.choose(ci, [c[2] for c in CHUNKS])
    Wc = np.choose(ci, [c[3] for c in CHUNKS])
    pk = np.choose(ci, [c[4] for c in CHUNKS])
    off = np.choose(ci, FOFFS)
    w = X - xw0                            # [2D, 1, W]
    Hh = np.arange(H)[None, :, None]
    qq, hh = Hh // HQ, Hh % HQ
    g, j = hh // pk, hh % pk
    part = (128 // pk) * j + (u - u0)      # pack=1 -> j==0
    gidx = qq * (128 * FREE) + part * FREE + off + g * Wc + w
    _CACHE["idx"] = np.ascontiguousarray(gidx.astype(np.int64))
    return _CACHE["idx"]


def _assemble(stag_b):
    """stag_b: packed band (bf16 or mixed bf16/fp8 bytes) -> [2D,H,W] f32"""
    import ml_dtypes
    idx = _gather_idx()
    arr = np.asarray(stag_b)
    if arr.dtype == np.uint8:
        a3 = np.ascontiguousarray(arr.reshape(4, 128, BTOT))
        full = np.empty((4, 128, FREE), np.float32)
        for ci, (_u0, _U, _x0, _W, _p) in enumerate(CHUNKS):
            n = (HQ // _p) * _W
            dt = ml_dtypes.float8_e4m3 if F8[ci] else ml_dtypes.bfloat16
            full[:, :, FOFFS[ci]:FOFFS[ci] + n] = (
                a3[:, :, BOFFS[ci]:BOFFS[ci] + n * ESZ[ci]]
                .view(dt).astype(np.float32))
        flat = full.ravel()
    else:
        flat = arr.astype(np.float32).ravel()
    o = np.take(flat, idx)
    for d in range(1, D):
        o[d, :, :d] = 0
        o[D + d, :, WIMG - d:] = 0
    return o


def run_cores(right_np, left_np, timing_reps=0):
    """Run the SPMD bass kernel; returns list of per-core staging arrays."""
    import ml_dtypes
    from concourse.bass_utils import run_bass_kernel_spmd

    nc = _get_nc()
    bf = ml_dtypes.bfloat16
    in_maps = [
        {"r_in": np.ascontiguousarray(right_np[b].astype(bf)),
         "l_in": np.ascontiguousarray(left_np[b].astype(bf))}
        for b in range(B)
    ]
    res = run_bass_kernel_spmd(nc, in_maps, list(range(B)))
    return [res.results[b]["stag"] for b in range(B)]


def kernel(right_feature, left_feature, max_disp):
    assert int(max_disp) == D
    right_np = np.asarray(right_feature, dtype=np.float32)
    left_np = np.asarray(left_feature, dtype=np.float32)
    stags = run_cores(right_np, left_np)
    out = np.stack([_assemble(s) for s in stags])
    return out


# revision 33
# speedup vs baseline: 1.0001x; 1.0001x over previous
"""Bi-directional correlation cost volume on 8 Trainium2 NeuronCores.

Strategy (data-parallel over batch, one batch element per core):
  - Host casts both feature maps to bf16 (rel err ~3e-3, gate is 2e-2).
    The PE computes the Gram band G[u, x] = sum_c L[c,h,u] * R[c,h,x]
    at bf16 rate, 4x h-quadrant row-tiled (K=32 each) via tile_position.
  - The u-range [0,320) is covered by chunks of U in {128, 64}. U=64
    chunks pack TWO h-rows per PSUM tile (partitions 0-63 / 64-127 via
    tile_position col-tile 0/64), so every staging tile is a full 128
    partitions. That (a) halves those chunks' drain columns and (b)
    makes the whole quadrant's band one dense [128, FREE] SBUF tile.
  - Stores: ONE ~4.1MB DMA per quadrant (4/rep). Measured HWDGE-ring
    cost is ~1us fixed per dma_start + bytes at ~351 GB/s; the V1
    design's 24 stores/rep paid ~24us/rep of fixed cost alone.
  - KROT=2: input partitions hold (h%4, c) so consecutive matmuls cycle
    all 4 PE row-groups; with bank-interleaved PSUM slot offsets
    (concurrent row tiles must hit distinct banks) the PE subarray
    tiles overlap ~1.5x, dropping PE from 54.9us serial to ~37us -
    below the store wall.
  - PSUM (f32) drains to SBUF as bf16 with the 1/C scale fused, batched
    GB=4 matmuls per instruction, split across DVE and ACT by greedy
    load balance (~2.16 cols/ns combined).
  - Default cover (KCH=k5f8): five U=64 chunks (52736 band elems per
    h-row), with the two edge chunks staged as fp8e4 and decoded on
    host: 30.8% of elements at ~2.6% RMS quantization -> rel err
    1.556e-2 vs the 2e-2 gate, and staged bytes drop to 14.28MB/rep
    (27880 B/partition quadrant stores, under the ~33KB DMA-rate
    cliff). Measured 49.0us/rep, PE/drain/store co-walled; history:
    77.9 (V1, 24 stores) -> 56.6 (k5p single-store) -> 52.9 (KROT=2)
    -> 50.6 (fp8 edges) -> 49.0 (UNROLL=32 For_i amortization).
  - The cost volume out[d, x] = G[x -/+ d, x] is a shear of the band;
    host extracts the diagonals with one vectorized gather per batch.
"""

import os

import numpy as np

B, C, H, WIMG, D = 8, 32, 160, 320, 64
HQ = H // 4      # 40 h-rows per PE quadrant

# (u0, U, xw0, W, pack): u-chunk start/size, x-window start/size,
# h-rows packed per PSUM partition-group (pack = 128 // U).
_KCH = os.environ.get("KCH", "k5f8")
if _KCH == "k3p":
    CHUNKS = [(0, 128, 0, 191, 1), (128, 64, 65, 190, 2),
              (192, 128, 129, 191, 1)]
elif _KCH == "k4p":
    CHUNKS = [(0, 128, 0, 191, 1), (128, 64, 65, 190, 2),
              (192, 64, 129, 190, 2), (256, 64, 193, 127, 2)]
elif _KCH == "k4f8":
    # k4p geometry (698 PE cols/h-row vs k5p's 824 - the PE stream is
    # the wall once fp8 pays for the bytes) with chunks c1+c3 staged
    # fp8e4: 35.7% of elements at ~2.6% RMS -> total rel err ~1.7e-2;
    # 14.96MB/rep staged, 29220 B/partition quadrant stores.
    CHUNKS = [(0, 128, 0, 191, 1), (128, 64, 65, 190, 2),
              (192, 64, 129, 190, 2), (256, 64, 193, 127, 2)]
elif _KCH == "k5f8":
    # k5p geometry, but the two edge chunks staged as fp8e4 (the host
    # decodes them back to f32): 30.8% of band elements at ~2.6% RMS
    # quantization -> total rel err ~1.5e-2 (gate 2e-2), staged bytes
    # drop 15.4% to 27880 B/partition per quadrant store.
    CHUNKS = [(0, 64, 0, 127, 2), (64, 64, 1, 190, 2),
              (128, 64, 65, 190, 2), (192, 64, 129, 190, 2),
              (256, 64, 193, 127, 2)]
elif _KCH == "k6m":
    # four U=64 + two U=32 chunks: 51712 elems/h-row (-2% vs k5p),
    # 32320 B/partition stores; PE 61.3us serial needs KROT=2.
    CHUNKS = [(0, 64, 0, 127, 2), (64, 64, 1, 190, 2),
              (128, 64, 65, 190, 2), (192, 64, 129, 190, 2),
              (256, 32, 193, 127, 4), (288, 32, 225, 95, 4)]
elif _KCH == "k10":
    # all-U=32 cover, pack=4 (needs KROT=2 PE tile concurrency to keep
    # the 92.8us serial PE stream under the ~47us store wall): 44544
    # band elems/h-row -> 14.25MB/rep, stores 27840 B/partition.
    CHUNKS = [(0, 32, 0, 95, 4), (32, 32, 0, 127, 4),
              (64, 32, 1, 158, 4), (96, 32, 33, 158, 4),
              (128, 32, 65, 158, 4), (160, 32, 97, 158, 4),
              (192, 32, 129, 158, 4), (224, 32, 161, 158, 4),
              (256, 32, 193, 127, 4), (288, 32, 225, 95, 4)]
else:
    # all-U=64 cover: fewest staged bytes (52736 elems/h-row) AND a
    # per-quadrant store of 32960 B/partition, which sits below the
    # ~33KB/partition DMA-rate cliff (351 GB/s vs 306 above it).
    CHUNKS = [(0, 64, 0, 127, 2), (64, 64, 1, 190, 2),
              (128, 64, 65, 190, 2), (192, 64, 129, 190, 2),
              (256, 64, 193, 127, 2)]

# staging free-dim offsets: chunk ci occupies [FOFF, FOFF+(HQ//pack)*W)
# elements; F8[ci] chunks are staged as fp8e4 (1B), the rest bf16 (2B).
F8 = {"k5f8": [True, False, False, False, True],
      "k4f8": [False, True, False, True]}.get(_KCH, [False] * len(CHUNKS))
ESZ = [1 if f else 2 for f in F8]
FOFFS, FREE = [], 0          # element offsets (host-side f32 array)
BOFFS, BTOT = [], 0          # byte offsets (device staging layout)
for _ci, (_u0, _U, _x0, _W, _p) in enumerate(CHUNKS):
    FOFFS.append(FREE)
    FREE += (HQ // _p) * _W
    BOFFS.append(BTOT)
    BTOT += (HQ // _p) * _W * ESZ[_ci]
# Store-DMA rate depends non-monotonically on bytes/partition: 27880
# (k5f8's natural size) sustains only ~302 GB/s while 30560 measures
# ~346. Pad the staging tile/store with junk bytes up to the nearest
# measured-good shape: +1.37MB/rep of pad beats the rate cliff by ~3us.
BSTORE = 30560 if BTOT == 27880 else BTOT

_CACHE = {}

# tuning knobs (env-overridable for experiments)
GB = int(os.environ.get("KGB", "4"))         # matmuls per PSUM tile/copy
# PE row-band rotation (partition (h%4, c)) so consecutive matmuls hit
# distinct PE row-groups -> subarray tile concurrency (~1.5x measured).
# Concurrent row tiles MUST write distinct PSUM banks: KROT=2 keeps the
# standard 4x2-bank PSUM tiles but bank-interleaves slot offsets
# (0,512,256,768) so every same-col-half pair in the concurrency window
# differs in bank. (KROT=1, a 2x4-bank spread, starves PSUM
# double-buffering and is slower; KROT=0 is the serial-PE layout.)
KROT = int(os.environ.get("KROT", "2"))
KPE2 = int(os.environ.get("KPE2", "0"))      # issue each matmul twice (probe)
PS_BUFS = int(os.environ.get("KPSB", "4"))   # PSUM pool buffers
ST_BUFS = int(os.environ.get("KSTB", "3"))   # staging pool buffers
UNROLL = int(os.environ.get("KUNR", "32"))   # reps per For_i iteration
KF8S = int(os.environ.get("KF8S", "0"))      # force fp8 drains onto ACT
NDMAQ = int(os.environ.get("KDMA", "1"))     # store-DMA queues


def _get_nc(reps=1, hw_loop=False):
    """reps identical kernel bodies; with hw_loop, a For_i loop of
    reps//UNROLL iterations around an UNROLL-times unrolled body (constant
    NEFF size, so huge rep counts stay compilable - used for timing)."""
    key = ("nc", reps, GB, PS_BUFS, ST_BUFS, _KCH, hw_loop, UNROLL, NDMAQ,
           KROT, KPE2, KF8S)
    if key in _CACHE:
        return _CACHE[key]
    import concourse.bacc as bacc
    import concourse.tile as tile
    from concourse import mybir

    f32 = mybir.dt.float32
    bf16 = mybir.dt.bfloat16
    nc = bacc.Bacc("TRN2", target_bir_lowering=False, debug=False)
    r_in = nc.declare_dram_parameter("r_in", [C, H, WIMG], bf16, isOutput=False)
    l_in = nc.declare_dram_parameter("l_in", [C, H, WIMG], bf16, isOutput=False)
    u8 = mybir.dt.uint8
    f8 = mybir.dt.float8e4
    stag = nc.declare_dram_parameter("stag", [4, 128 * BSTORE], u8,
                                     isOutput=True)

    # KROT=1: 2 x 4-bank tiles = full PSUM; KROT=2: standard 4 x 2-bank
    ps_bufs = 2 if KROT == 1 else PS_BUFS
    with tile.TileContext(nc) as tc:
        with tc.tile_pool(name="inp", bufs=1) as inp_pool, \
             tc.tile_pool(name="ps", bufs=ps_bufs, space="PSUM") as ps_pool, \
             tc.tile_pool(name="st", bufs=ST_BUFS) as st_pool:
            Lsb = inp_pool.tile([128, HQ * WIMG], bf16, tag="L")
            Rsb = inp_pool.tile([128, HQ * WIMG], bf16, tag="R")
            if KROT:
                # partition (h%4, c) holds h-row h at free slot h//4: the
                # mm stream then cycles all 4 PE row-groups (tile
                # concurrency), while the PSUM/staging layout is unchanged.
                for s in range(4):
                    nc.sync.dma_start(
                        Lsb[32 * s:32 * (s + 1), :].rearrange(
                            "p (hh x) -> p hh x", x=WIMG),
                        l_in[:, s::4, :])
                    nc.sync.dma_start(
                        Rsb[32 * s:32 * (s + 1), :].rearrange(
                            "p (hh x) -> p hh x", x=WIMG),
                        r_in[:, s::4, :])
            else:
                # partition (q, c) holds h-rows [40q, 40q+40) of channel c
                for q in range(4):
                    nc.sync.dma_start(
                        Lsb[32 * q:32 * (q + 1), :],
                        l_in[:, HQ * q:HQ * (q + 1), :].rearrange(
                            "c hh x -> c (hh x)"),
                    )
                    nc.sync.dma_start(
                        Rsb[32 * q:32 * (q + 1), :],
                        r_in[:, HQ * q:HQ * (q + 1), :].rearrange(
                            "c hh x -> c (hh x)"),
                    )
            # greedy copy-engine balance: projected busy ns per engine
            # (per-instruction init: ACT 2*222cyc/2, DVE 2*120cyc/2 busy)
            eng_ns = {"v": 0.0, "s": 0.0}
            cyc = {"v": 1.0 / 0.96, "s": 1.0 / 1.2}
            init = {"v": 120 * cyc["v"], "s": 222 * cyc["s"]}

            def mm1(ps, q, g, kslot, ci):
                u0, U, xw0, W, pack = CHUNKS[ci]
                for j in range(pack):
                    hh = g * pack + j
                    if KROT:
                        s, fr = hh % 4, 10 * q + hh // 4
                    else:
                        s, fr = q, hh
                    for _rep in range(1 + KPE2):
                        nc.tensor.matmul(
                            ps[U * j:U * j + U,
                               256 * kslot:256 * kslot + W],
                            Lsb[32 * s:32 * (s + 1),
                                fr * WIMG + u0:fr * WIMG + u0 + U],
                            Rsb[32 * s:32 * (s + 1),
                                fr * WIMG + xw0:fr * WIMG + xw0 + W],
                            start=True, stop=True,
                            tile_position=(32 * s, U * j),
                        )

            # With KROT, group a of a batch uses PSUM slot perm(a) so that
            # consecutive (concurrently executing) row-rotated matmuls hit
            # distinct PSUM banks: slot(a) = (a%4)*2 + a//4 for nb=8 (banks
            # 0,1,2,3,0,1,2,3), slot(a) = 2a for nb<=4 (banks 0..3).
            def slot_of(a, nb):
                if not KROT:
                    return a
                if KROT == 2:
                    # bank-interleaved slots within a standard 2-bank tile:
                    # offsets (0, 512, 256, 768) f32 -> banks 0,1,0,1 so any
                    # two same-col-half mms within the concurrency window
                    # differ in bank (same-bank pairs are same-tile-position
                    # and serialize on weights anyway).
                    assert nb in (2, 4)
                    return (a % 2) * 2 + a // 2 if nb == 4 else 2 * a
                return (a % 4) * 2 + a // 4 if nb == 8 else 2 * a

            def drain(ps, sb, g0, nb, ci):
                _, U, _, W, pack = CHUNKS[ci]
                esz = ESZ[ci]
                dt_out = f8 if F8[ci] else bf16

                def dview(ng_):
                    b0 = BOFFS[ci] + g0 * W * esz
                    return sb[:, b0:b0 + ng_ * W * esz].bitcast(dt_out)

                if not KROT:
                    src = ps[:, :256 * nb].rearrange(
                        "u (g c) -> u g c", g=nb)[:, :, :W]
                    dst = dview(nb).rearrange("u (g w) -> u g w", g=nb)
                elif KROT == 2 and nb == 4:
                    # slot offset = (a%2)*512 + (a//2)*256: a = 2*hi + lo
                    src = ps[:, :1024].rearrange(
                        "u (lo hi c) -> u hi lo c", lo=2, hi=2)[:, :, :, :W]
                    dst = dview(4).rearrange("u (hi lo w) -> u hi lo w",
                                             hi=2, lo=2)
                elif KROT == 2:
                    # tail: slots (0, 2) -> offsets (0, 512)
                    assert nb == 2
                    src = ps[:, :1024].rearrange(
                        "u (lo c) -> u lo c", lo=2)[:, :, :W]
                    dst = dview(2).rearrange("u (g w) -> u g w", g=2)
                elif nb == 8:
                    # slot offset = (a%4)*512 + (a//4)*256: a = 4*hi + lo
                    src = ps[:, :2048].rearrange(
                        "u (lo hi c) -> u hi lo c", lo=4, hi=2)[:, :, :, :W]
                    dst = dview(8).rearrange("u (hi lo w) -> u hi lo w",
                                             hi=2, lo=4)
                else:
                    assert nb <= 4
                    src = ps[:, :512 * nb].rearrange(
                        "u (g c) -> u g c", g=nb)[:, :, :W]
                    dst = dview(nb).rearrange("u (g w) -> u g w", g=nb)
                cost = {e: nb * W * cyc[e] + init[e] for e in ("v", "s")}
                if KF8S and F8[ci]:
                    e = "s"
                else:
                    e = min(("v", "s"), key=lambda x: eng_ns[x] + cost[x])
                eng_ns[e] += cost[e]
                if e == "s":
                    nc.scalar.mul(dst, src, 1.0 / C)
                else:
                    nc.vector.tensor_scalar_mul(dst, src, 1.0 / C)

            # 1: SP HWDGE only; 2: +ACT HWDGE; 3: +Pool SWDGE (idle engine,
            # parallel ring - measured ~3us/rep faster than SP alone)
            dmaq = {1: [nc.sync], 2: [nc.sync, nc.scalar],
                    3: [nc.sync, nc.gpsimd]}[NDMAQ]
            store_ct = [0]

            def store(sb, q):
                eng = dmaq[store_ct[0] % len(dmaq)]
                store_ct[0] += 1
                eng.dma_start(
                    stag[q].rearrange("(u k) -> u k", u=128), sb[:, :])

            gb_eff = GB
            ps_w = 2048 if KROT == 1 else 256 * GB  # KROT=1: 4-bank spread

            def rep_body():
                for q in range(4):
                    sb = st_pool.tile([128, BSTORE], u8, tag="sb")
                    for ci, (u0, U, xw0, W, pack) in enumerate(CHUNKS):
                        ng = HQ // pack
                        for g0 in range(0, ng, gb_eff):
                            nb = min(gb_eff, ng - g0)
                            ps = ps_pool.tile([128, ps_w], f32,
                                              tag="ps")
                            for k in range(nb):
                                mm1(ps, q, g0 + k, slot_of(k, nb), ci)
                            drain(ps, sb, g0, nb, ci)
                    store(sb, q)

            if hw_loop:
                assert reps % UNROLL == 0
                with tc.For_i(0, reps // UNROLL) as _iv:
                    for _ in range(UNROLL):
                        rep_body()
            else:
                for _ in range(reps):
                    rep_body()
    nc.compile()
    _CACHE[key] = nc
    return nc


def _gather_idx():
    """GIDX[p, h, x]: flat index into stag.ravel() for output plane p."""
    if "idx" in _CACHE:
        return _CACHE["idx"]
    P_ = np.arange(2 * D)[:, None, None]
    dts = np.where(P_ < D, P_, -(P_ - D))  # signed disparity per plane
    X = np.arange(WIMG)[None, None, :]
    u = np.clip(X - dts, 0, WIMG - 1)      # [2D, 1, W]
    ci = np.zeros(u.shape, dtype=np.int64)
    for i in range(1, len(CHUNKS)):
        ci += (u >= CHUNKS[i][0])
    u0 = np.choose(ci, [c[0] for c in CHUNKS])
    xw0 = np.choose(ci, [c[2] for c in CHUNKS])
    Wc = np.choose(ci, [c[3] for c in CHUNKS])
    pk = np.choose(ci, [c[4] for c in CHUNKS])
    off = np.choose(ci, FOFFS)
    w = X - xw0                            # [2D, 1, W]
    Hh = np.arange(H)[None, :, None]
    qq, hh = Hh // HQ, Hh % HQ
    g, j = hh // pk, hh % pk
    part = (128 // pk) * j + (u - u0)      # pack=1 -> j==0
    gidx = qq * (128 * FREE) + part * FREE + off + g * Wc + w
    _CACHE["idx"] = np.ascontiguousarray(gidx.astype(np.int64))
    return _CACHE["idx"]


def _assemble(stag_b):
    """stag_b: packed band (bf16 or mixed bf16/fp8 bytes) -> [2D,H,W] f32"""
    import ml_dtypes
    idx = _gather_idx()
    arr = np.asarray(stag_b)
    if arr.dtype == np.uint8:
        a3 = np.ascontiguousarray(arr.reshape(4, 128, BSTORE))
        full = np.empty((4, 128, FREE), np.float32)
        for ci, (_u0, _U, _x0, _W, _p) in enumerate(CHUNKS):
            n = (HQ // _p) * _W
            dt = ml_dtypes.float8_e4m3 if F8[ci] else ml_dtypes.bfloat16
            full[:, :, FOFFS[ci]:FOFFS[ci] + n] = (
                a3[:, :, BOFFS[ci]:BOFFS[ci] + n * ESZ[ci]]
                .view(dt).astype(np.float32))
        flat = full.ravel()
    else:
        flat = arr.astype(np.float32).ravel()
    o = np.take(flat, idx)
    for d in range(1, D):
        o[d, :, :d] = 0
        o[D + d, :, WIMG - d:] = 0
    return o


def run_cores(right_np, left_np, timing_reps=0):
    """Run the SPMD bass kernel; returns list of per-core staging arrays."""
    import ml_dtypes
    from concourse.bass_utils import run_bass_kernel_spmd

    nc = _get_nc()
    bf = ml_dtypes.bfloat16
    in_maps = [
        {"r_in": np.ascontiguousarray(right_np[b].astype(bf)),
         "l_in": np.ascontiguousarray(left_np[b].astype(bf))}
        for b in range(B)
    ]
    res = run_bass_kernel_spmd(nc, in_maps, list(range(B)))
    return [res.results[b]["stag"] for b in range(B)]


def kernel(right_feature, left_feature, max_disp):
    assert int(max_disp) == D
    right_np = np.asarray(right_feature, dtype=np.float32)
    left_np = np.asarray(left_feature, dtype=np.float32)
    stags = run_cores(right_np, left_np)
    out = np.stack([_assemble(s) for s in stags])
    return out


# revision 35
# speedup vs baseline: 1.0329x; 1.0328x over previous
"""Bi-directional correlation cost volume on 8 Trainium2 NeuronCores.

Strategy (data-parallel over batch, one batch element per core):
  - Host casts both feature maps to bf16 (rel err ~3e-3, gate is 2e-2).
    The PE computes the Gram band G[u, x] = sum_c L[c,h,u] * R[c,h,x]
    at bf16 rate, 4x h-quadrant row-tiled (K=32 each) via tile_position.
  - The u-range [0,320) is covered by chunks of U in {128, 64}. U=64
    chunks pack TWO h-rows per PSUM tile (partitions 0-63 / 64-127 via
    tile_position col-tile 0/64), so every staging tile is a full 128
    partitions. That (a) halves those chunks' drain columns and (b)
    makes the whole quadrant's band one dense [128, FREE] SBUF tile.
  - Stores: ONE ~4.1MB DMA per quadrant (4/rep). Measured HWDGE-ring
    cost is ~1us fixed per dma_start + bytes at ~351 GB/s; the V1
    design's 24 stores/rep paid ~24us/rep of fixed cost alone.
  - KROT=2: input partitions hold (h%4, c) so consecutive matmuls cycle
    all 4 PE row-groups; with bank-interleaved PSUM slot offsets
    (concurrent row tiles must hit distinct banks) the PE subarray
    tiles overlap ~1.5x, dropping PE from 54.9us serial to ~37us -
    below the store wall.
  - PSUM (f32) drains to SBUF as bf16 with the 1/C scale fused, batched
    GB=4 matmuls per instruction, split across DVE and ACT by greedy
    load balance (~2.16 cols/ns combined).
  - Default cover (KCH=k5f8): five U=64 chunks (52736 band elems per
    h-row), with the two edge chunks staged as fp8e4 and decoded on
    host: 30.8% of elements at ~2.6% RMS quantization -> rel err
    1.556e-2 vs the 2e-2 gate, and staged bytes drop to 14.28MB/rep
    (27880 B/partition quadrant stores, under the ~33KB DMA-rate
    cliff). Measured 49.0us/rep, PE/drain/store co-walled; history:
    77.9 (V1, 24 stores) -> 56.6 (k5p single-store) -> 52.9 (KROT=2)
    -> 50.6 (fp8 edges) -> 49.0 (UNROLL=32 For_i amortization).
  - The cost volume out[d, x] = G[x -/+ d, x] is a shear of the band;
    host extracts the diagonals with one vectorized gather per batch.
"""

import os

import numpy as np

B, C, H, WIMG, D = 8, 32, 160, 320, 64
HQ = H // 4      # 40 h-rows per PE quadrant

# (u0, U, xw0, W, pack): u-chunk start/size, x-window start/size,
# h-rows packed per PSUM partition-group (pack = 128 // U).
_KCH = os.environ.get("KCH", "k5f8")
if _KCH == "k3p":
    CHUNKS = [(0, 128, 0, 191, 1), (128, 64, 65, 190, 2),
              (192, 128, 129, 191, 1)]
elif _KCH == "k4p":
    CHUNKS = [(0, 128, 0, 191, 1), (128, 64, 65, 190, 2),
              (192, 64, 129, 190, 2), (256, 64, 193, 127, 2)]
elif _KCH == "k4f8":
    # k4p geometry (698 PE cols/h-row vs k5p's 824 - the PE stream is
    # the wall once fp8 pays for the bytes) with chunks c1+c3 staged
    # fp8e4: 35.7% of elements at ~2.6% RMS -> total rel err ~1.7e-2;
    # 14.96MB/rep staged, 29220 B/partition quadrant stores.
    CHUNKS = [(0, 128, 0, 191, 1), (128, 64, 65, 190, 2),
              (192, 64, 129, 190, 2), (256, 64, 193, 127, 2)]
elif _KCH in ("k5f8", "k5f8h"):
    # k5p geometry, but the two edge chunks staged as fp8e4 (the host
    # decodes them back to f32): 30.8% of band elements at ~2.6% RMS
    # quantization -> total rel err ~1.5e-2 (gate 2e-2), staged bytes
    # drop 15.4% to 27880 B/partition per quadrant store.
    CHUNKS = [(0, 64, 0, 127, 2), (64, 64, 1, 190, 2),
              (128, 64, 65, 190, 2), (192, 64, 129, 190, 2),
              (256, 64, 193, 127, 2)]
elif _KCH == "k6m":
    # four U=64 + two U=32 chunks: 51712 elems/h-row (-2% vs k5p),
    # 32320 B/partition stores; PE 61.3us serial needs KROT=2.
    CHUNKS = [(0, 64, 0, 127, 2), (64, 64, 1, 190, 2),
              (128, 64, 65, 190, 2), (192, 64, 129, 190, 2),
              (256, 32, 193, 127, 4), (288, 32, 225, 95, 4)]
elif _KCH == "k10":
    # all-U=32 cover, pack=4 (needs KROT=2 PE tile concurrency to keep
    # the 92.8us serial PE stream under the ~47us store wall): 44544
    # band elems/h-row -> 14.25MB/rep, stores 27840 B/partition.
    CHUNKS = [(0, 32, 0, 95, 4), (32, 32, 0, 127, 4),
              (64, 32, 1, 158, 4), (96, 32, 33, 158, 4),
              (128, 32, 65, 158, 4), (160, 32, 97, 158, 4),
              (192, 32, 129, 158, 4), (224, 32, 161, 158, 4),
              (256, 32, 193, 127, 4), (288, 32, 225, 95, 4)]
else:
    # all-U=64 cover: fewest staged bytes (52736 elems/h-row) AND a
    # per-quadrant store of 32960 B/partition, which sits below the
    # ~33KB/partition DMA-rate cliff (351 GB/s vs 306 above it).
    CHUNKS = [(0, 64, 0, 127, 2), (64, 64, 1, 190, 2),
              (128, 64, 65, 190, 2), (192, 64, 129, 190, 2),
              (256, 64, 193, 127, 2)]

# staging free-dim offsets: chunk ci occupies [FOFF, FOFF+(HQ//pack)*W)
# elements; F8[ci] chunks are staged as fp8e4 (1B), the rest bf16 (2B).
F8 = {"k5f8": [True, False, False, False, True],
      "k5f8h": [True, False, False, False, True],
      "k4f8": [False, True, False, True]}.get(_KCH, [False] * len(CHUNKS))
ESZ = [1 if f else 2 for f in F8]
FOFFS, FREE = [], 0          # element offsets (host-side f32 array)
for _ci, (_u0, _U, _x0, _W, _p) in enumerate(CHUNKS):
    FOFFS.append(FREE)
    FREE += (HQ // _p) * _W
# Staging sub-regions (ci, g0, ngroups, is_fp8): normally one per chunk,
# but k5f8h additionally stages chunk 1's first 8 h-groups (of 20) as
# fp8 - 40% of elements total at ~2.6% RMS -> rel err ~1.74e-2.
SUBREGS = []
for _ci, (_u0, _U, _x0, _W, _p) in enumerate(CHUNKS):
    _ng = HQ // _p
    if _KCH == "k5f8h" and _ci == 1:
        SUBREGS += [(_ci, 0, 8, True), (_ci, 8, _ng - 8, False)]
    else:
        SUBREGS.append((_ci, 0, _ng, F8[_ci]))
SUBB, BTOT = [], 0           # byte offset per subregion
for (_ci, _g0, _ngs, _f8) in SUBREGS:
    SUBB.append(BTOT)
    BTOT += _ngs * CHUNKS[_ci][3] * (1 if _f8 else 2)
# (Padding the store up to a better-measured pure-store shape (30560
# B/partition, ~346 GB/s vs ~302 at 27880) was tried and is timing-
# neutral in-kernel: the store is not the binding rate here.)
BSTORE = BTOT

_CACHE = {}

# tuning knobs (env-overridable for experiments)
GB = int(os.environ.get("KGB", "4"))         # matmuls per PSUM tile/copy
# PE row-band rotation (partition (h%4, c)) so consecutive matmuls hit
# distinct PE row-groups -> subarray tile concurrency (~1.5x measured).
# Concurrent row tiles MUST write distinct PSUM banks: KROT=2 keeps the
# standard 4x2-bank PSUM tiles but bank-interleaves slot offsets
# (0,512,256,768) so every same-col-half pair in the concurrency window
# differs in bank. (KROT=1, a 2x4-bank spread, starves PSUM
# double-buffering and is slower; KROT=0 is the serial-PE layout.)
KROT = int(os.environ.get("KROT", "2"))
KPE2 = int(os.environ.get("KPE2", "0"))      # issue each matmul twice (probe)
PS_BUFS = int(os.environ.get("KPSB", "4"))   # PSUM pool buffers
ST_BUFS = int(os.environ.get("KSTB", "3"))   # staging pool buffers
UNROLL = int(os.environ.get("KUNR", "32"))   # reps per For_i iteration
KF8S = int(os.environ.get("KF8S", "0"))      # force fp8 drains onto ACT
NDMAQ = int(os.environ.get("KDMA", "1"))     # store-DMA queues


def _get_nc(reps=1, hw_loop=False):
    """reps identical kernel bodies; with hw_loop, a For_i loop of
    reps//UNROLL iterations around an UNROLL-times unrolled body (constant
    NEFF size, so huge rep counts stay compilable - used for timing)."""
    key = ("nc", reps, GB, PS_BUFS, ST_BUFS, _KCH, hw_loop, UNROLL, NDMAQ,
           KROT, KPE2, KF8S)
    if key in _CACHE:
        return _CACHE[key]
    import concourse.bacc as bacc
    import concourse.tile as tile
    from concourse import mybir

    f32 = mybir.dt.float32
    bf16 = mybir.dt.bfloat16
    nc = bacc.Bacc("TRN2", target_bir_lowering=False, debug=False)
    r_in = nc.declare_dram_parameter("r_in", [C, H, WIMG], bf16, isOutput=False)
    l_in = nc.declare_dram_parameter("l_in", [C, H, WIMG], bf16, isOutput=False)
    u8 = mybir.dt.uint8
    f8 = mybir.dt.float8e4
    stag = nc.declare_dram_parameter("stag", [4, 128 * BSTORE], u8,
                                     isOutput=True)

    # KROT=1: 2 x 4-bank tiles = full PSUM; KROT=2: standard 4 x 2-bank
    ps_bufs = 2 if KROT == 1 else PS_BUFS
    with tile.TileContext(nc) as tc:
        with tc.tile_pool(name="inp", bufs=1) as inp_pool, \
             tc.tile_pool(name="ps", bufs=ps_bufs, space="PSUM") as ps_pool, \
             tc.tile_pool(name="st", bufs=ST_BUFS) as st_pool:
            Lsb = inp_pool.tile([128, HQ * WIMG], bf16, tag="L")
            Rsb = inp_pool.tile([128, HQ * WIMG], bf16, tag="R")
            if KROT:
                # partition (h%4, c) holds h-row h at free slot h//4: the
                # mm stream then cycles all 4 PE row-groups (tile
                # concurrency), while the PSUM/staging layout is unchanged.
                for s in range(4):
                    nc.sync.dma_start(
                        Lsb[32 * s:32 * (s + 1), :].rearrange(
                            "p (hh x) -> p hh x", x=WIMG),
                        l_in[:, s::4, :])
                    nc.sync.dma_start(
                        Rsb[32 * s:32 * (s + 1), :].rearrange(
                            "p (hh x) -> p hh x", x=WIMG),
                        r_in[:, s::4, :])
            else:
                # partition (q, c) holds h-rows [40q, 40q+40) of channel c
                for q in range(4):
                    nc.sync.dma_start(
                        Lsb[32 * q:32 * (q + 1), :],
                        l_in[:, HQ * q:HQ * (q + 1), :].rearrange(
                            "c hh x -> c (hh x)"),
                    )
                    nc.sync.dma_start(
                        Rsb[32 * q:32 * (q + 1), :],
                        r_in[:, HQ * q:HQ * (q + 1), :].rearrange(
                            "c hh x -> c (hh x)"),
                    )
            # greedy copy-engine balance: projected busy ns per engine
            # (per-instruction init: ACT 2*222cyc/2, DVE 2*120cyc/2 busy)
            eng_ns = {"v": 0.0, "s": 0.0}
            cyc = {"v": 1.0 / 0.96, "s": 1.0 / 1.2}
            init = {"v": 120 * cyc["v"], "s": 222 * cyc["s"]}

            def mm1(ps, q, g, kslot, ci):
                u0, U, xw0, W, pack = CHUNKS[ci]
                for j in range(pack):
                    hh = g * pack + j
                    if KROT:
                        s, fr = hh % 4, 10 * q + hh // 4
                    else:
                        s, fr = q, hh
                    for _rep in range(1 + KPE2):
                        nc.tensor.matmul(
                            ps[U * j:U * j + U,
                               256 * kslot:256 * kslot + W],
                            Lsb[32 * s:32 * (s + 1),
                                fr * WIMG + u0:fr * WIMG + u0 + U],
                            Rsb[32 * s:32 * (s + 1),
                                fr * WIMG + xw0:fr * WIMG + xw0 + W],
                            start=True, stop=True,
                            tile_position=(32 * s, U * j),
                        )

            # With KROT, group a of a batch uses PSUM slot perm(a) so that
            # consecutive (concurrently executing) row-rotated matmuls hit
            # distinct PSUM banks: slot(a) = (a%4)*2 + a//4 for nb=8 (banks
            # 0,1,2,3,0,1,2,3), slot(a) = 2a for nb<=4 (banks 0..3).
            def slot_of(a, nb):
                if not KROT:
                    return a
                if KROT == 2:
                    # bank-interleaved slots within a standard 2-bank tile:
                    # offsets (0, 512, 256, 768) f32 -> banks 0,1,0,1 so any
                    # two same-col-half mms within the concurrency window
                    # differ in bank (same-bank pairs are same-tile-position
                    # and serialize on weights anyway).
                    assert nb in (2, 4)
                    return (a % 2) * 2 + a // 2 if nb == 4 else 2 * a
                return (a % 4) * 2 + a // 4 if nb == 8 else 2 * a

            def drain(ps, sb, g0, nb, ci):
                _, U, _, W, pack = CHUNKS[ci]
                sri = next(i for i, (c, gs, ns_, _f) in enumerate(SUBREGS)
                           if c == ci and gs <= g0 < gs + ns_)
                _c, _gs, _ns, _isf8 = SUBREGS[sri]
                esz = 1 if _isf8 else 2
                dt_out = f8 if _isf8 else bf16

                def dview(ng_):
                    b0 = SUBB[sri] + (g0 - _gs) * W * esz
                    return sb[:, b0:b0 + ng_ * W * esz].bitcast(dt_out)

                if not KROT:
                    src = ps[:, :256 * nb].rearrange(
                        "u (g c) -> u g c", g=nb)[:, :, :W]
                    dst = dview(nb).rearrange("u (g w) -> u g w", g=nb)
                elif KROT == 2 and nb == 4:
                    # slot offset = (a%2)*512 + (a//2)*256: a = 2*hi + lo
                    src = ps[:, :1024].rearrange(
                        "u (lo hi c) -> u hi lo c", lo=2, hi=2)[:, :, :, :W]
                    dst = dview(4).rearrange("u (hi lo w) -> u hi lo w",
                                             hi=2, lo=2)
                elif KROT == 2:
                    # tail: slots (0, 2) -> offsets (0, 512)
                    assert nb == 2
                    src = ps[:, :1024].rearrange(
                        "u (lo c) -> u lo c", lo=2)[:, :, :W]
                    dst = dview(2).rearrange("u (g w) -> u g w", g=2)
                elif nb == 8:
                    # slot offset = (a%4)*512 + (a//4)*256: a = 4*hi + lo
                    src = ps[:, :2048].rearrange(
                        "u (lo hi c) -> u hi lo c", lo=4, hi=2)[:, :, :, :W]
                    dst = dview(8).rearrange("u (hi lo w) -> u hi lo w",
                                             hi=2, lo=4)
                else:
                    assert nb <= 4
                    src = ps[:, :512 * nb].rearrange(
                        "u (g c) -> u g c", g=nb)[:, :, :W]
                    dst = dview(nb).rearrange("u (g w) -> u g w", g=nb)
                cost = {e: nb * W * cyc[e] + init[e] for e in ("v", "s")}
                if KF8S and _isf8:
                    e = "s"
                else:
                    e = min(("v", "s"), key=lambda x: eng_ns[x] + cost[x])
                eng_ns[e] += cost[e]
                if e == "s":
                    nc.scalar.mul(dst, src, 1.0 / C)
                else:
                    nc.vector.tensor_scalar_mul(dst, src, 1.0 / C)

            # 1: SP HWDGE only; 2: +ACT HWDGE; 3: +Pool SWDGE (idle engine,
            # parallel ring - measured ~3us/rep faster than SP alone)
            dmaq = {1: [nc.sync], 2: [nc.sync, nc.scalar],
                    3: [nc.sync, nc.gpsimd]}[NDMAQ]
            store_ct = [0]

            def store(sb, q):
                eng = dmaq[store_ct[0] % len(dmaq)]
                store_ct[0] += 1
                eng.dma_start(
                    stag[q].rearrange("(u k) -> u k", u=128), sb[:, :])

            gb_eff = GB
            ps_w = 2048 if KROT == 1 else 256 * GB  # KROT=1: 4-bank spread

            def rep_body():
                for q in range(4):
                    sb = st_pool.tile([128, BSTORE], u8, tag="sb")
                    for ci, (u0, U, xw0, W, pack) in enumerate(CHUNKS):
                        ng = HQ // pack
                        for g0 in range(0, ng, gb_eff):
                            nb = min(gb_eff, ng - g0)
                            ps = ps_pool.tile([128, ps_w], f32,
                                              tag="ps")
                            for k in range(nb):
                                mm1(ps, q, g0 + k, slot_of(k, nb), ci)
                            drain(ps, sb, g0, nb, ci)
                    store(sb, q)

            if hw_loop:
                assert reps % UNROLL == 0
                with tc.For_i(0, reps // UNROLL) as _iv:
                    for _ in range(UNROLL):
                        rep_body()
            else:
                for _ in range(reps):
                    rep_body()
    nc.compile()
    _CACHE[key] = nc
    return nc


def _gather_idx():
    """GIDX[p, h, x]: flat index into stag.ravel() for output plane p."""
    if "idx" in _CACHE:
        return _CACHE["idx"]
    P_ = np.arange(2 * D)[:, None, None]
    dts = np.where(P_ < D, P_, -(P_ - D))  # signed disparity per plane
    X = np.arange(WIMG)[None, None, :]
    u = np.clip(X - dts, 0, WIMG - 1)      # [2D, 1, W]
    ci = np.zeros(u.shape, dtype=np.int64)
    for i in range(1, len(CHUNKS)):
        ci += (u >= CHUNKS[i][0])
    u0 = np.choose(ci, [c[0] for c in CHUNKS])
    xw0 = np.choose(ci, [c[2] for c in CHUNKS])
    Wc = np.choose(ci, [c[3] for c in CHUNKS])
    pk = np.choose(ci, [c[4] for c in CHUNKS])
    off = np.choose(ci, FOFFS)
    w = X - xw0                            # [2D, 1, W]
    Hh = np.arange(H)[None, :, None]
    qq, hh = Hh // HQ, Hh % HQ
    g, j = hh // pk, hh % pk
    part = (128 // pk) * j + (u - u0)      # pack=1 -> j==0
    gidx = qq * (128 * FREE) + part * FREE + off + g * Wc + w
    _CACHE["idx"] = np.ascontiguousarray(gidx.astype(np.int64))
    return _CACHE["idx"]


def _assemble(stag_b):
    """stag_b: packed band (bf16 or mixed bf16/fp8 bytes) -> [2D,H,W] f32"""
    import ml_dtypes
    idx = _gather_idx()
    arr = np.asarray(stag_b)
    if arr.dtype == np.uint8:
        a3 = np.ascontiguousarray(arr.reshape(4, 128, BSTORE))
        full = np.empty((4, 128, FREE), np.float32)
        for sri, (ci, g0, ngs, isf8) in enumerate(SUBREGS):
            W_ = CHUNKS[ci][3]
            n = ngs * W_
            e0 = FOFFS[ci] + g0 * W_
            dt = ml_dtypes.float8_e4m3 if isf8 else ml_dtypes.bfloat16
            full[:, :, e0:e0 + n] = (
                a3[:, :, SUBB[sri]:SUBB[sri] + n * (1 if isf8 else 2)]
                .view(dt).astype(np.float32))
        flat = full.ravel()
    else:
        flat = arr.astype(np.float32).ravel()
    o = np.take(flat, idx)
    for d in range(1, D):
        o[d, :, :d] = 0
        o[D + d, :, WIMG - d:] = 0
    return o


def run_cores(right_np, left_np, timing_reps=0):
    """Run the SPMD bass kernel; returns list of per-core staging arrays."""
    import ml_dtypes
    from concourse.bass_utils import run_bass_kernel_spmd

    nc = _get_nc()
    bf = ml_dtypes.bfloat16
    in_maps = [
        {"r_in": np.ascontiguousarray(right_np[b].astype(bf)),
         "l_in": np.ascontiguousarray(left_np[b].astype(bf))}
        for b in range(B)
    ]
    res = run_bass_kernel_spmd(nc, in_maps, list(range(B)))
    return [res.results[b]["stag"] for b in range(B)]


def kernel(right_feature, left_feature, max_disp):
    assert int(max_disp) == D
    right_np = np.asarray(right_feature, dtype=np.float32)
    left_np = np.asarray(left_feature, dtype=np.float32)
    stags = run_cores(right_np, left_np)
    out = np.stack([_assemble(s) for s in stags])
    return out
